# revision 35
# baseline (speedup 1.0000x reference)
"""Trainium2 Bass kernel for nn_MultiHeadAttention_56066503082144.

Reference computation (per batch b):
  Q = relu(x @ Wq + bq), K = relu(x @ Wk + bk), V = relu(x @ Wv + bv)
  scores[b,h,q,k] = (Q_h @ K_h^T) / sqrt(dh)
  attn = softmax(scores, axis=q)            # NON-STANDARD: over the query axis
  out[b,q,:] = concat_h(attn_h @ V_h)
  y = out + x                               # residual
  y = batchnorm(y)                          # per-channel stats over (B, S)

Sharding: data-parallel over batch B=8 across the 8 NeuronCores (one batch
element per core).  Cross-core communication: four 1KB AllReduces of
per-d-chunk BatchNorm partial sums, pipelined under the attention compute.

v2 design notes (trace-driven):
  - All projection/score matmuls run in float32r (TF32-like, 1 cycle/row at
    N=512) directly on f32 tiles - no bf16 weight casts at all.
  - Attention processes HEAD PAIRS: the two heads of a 128-partition Q/K tile
    occupy partitions 0:64 / 64:128, so their score matmuls auto-derive
    tile_position (0,0) / (64,0) (row groups) and run CONCURRENTLY on the PE's
    independent 64-row sub-arrays.  attn@V similarly col-tiles into po[0:64]
    and po[64:128] (tile_position (0,0) / (0,64)).
  - ScalarE is the pipeline pacer: 64 exp ACTIVATEs of [128,1024] (~1.02us
    each) with fused accum for softmax row sums.  Everything else is kept off
    ScalarE (K/V evacuation and the BN affine live on VectorE).
  - Softmax denominator folded into V rows (V' = relu(V+bv) * 1/rowsum as a
    single fused tensor_scalar) instead of rescaling the big E matrix.
  - BN tail pipelined per d-chunk m: residual+stats+AllReduce emitted with
    pair m, the post-collective ops (coeffs, affine, transpose-back, DMA out)
    emitted one pair LATER so in-order engine queues never stall on the
    collective.  Only chunk m=3's tail is exposed at the end.
  - rsd = exp(-0.5*ln(var+eps)): keeps every activation in the single
    natural_log_exp_and_others table set (no sqrt table load).  A dummy ln at
    kernel start pins that set.
"""

import math

import numpy as np

P = 128
D = 512
H = 8
DH = 64
S_FULL = 1024
B_FULL = 8
N_CORES = 8
BN_EPS = 1e-5

_CACHE = {}


class _Done(Exception):
    """Early-exit marker for phase-bisection builds."""


def _build(S=S_FULL, n_cores=N_CORES, total_tokens=None, stop_after="full",
           use_collective=True):
    import concourse.bacc as bacc
    import concourse.bass as bass
    import concourse.tile as tile
    from concourse import mybir
    from concourse.masks import make_identity

    f32 = mybir.dt.float32
    f32r = mybir.dt.float32r
    bf16 = mybir.dt.bfloat16
    AF = mybir.ActivationFunctionType
    ALU = mybir.AluOpType
    AX = mybir.AxisListType

    if total_tokens is None:
        total_tokens = n_cores * S
    inv_ntok = 1.0 / float(total_tokens)

    ND = D // P          # 4 d-chunks
    NS = S // P          # 8 s-chunks
    QW = min(512, S)     # matmul moving-operand tile
    NQ = S // QW         # 2
    NPAIR = H // 2       # 4 head pairs (pair hp lives in QT[hp]/KT[hp])
    inv_sqrt_dh = 1.0 / math.sqrt(DH)

    nc = bacc.Bacc(
        "TRN2",
        target_bir_lowering=False,
        debug=False,
        num_devices=n_cores,
    )

    x_d = nc.dram_tensor("x", [S, D], f32, kind="ExternalInput").ap()
    Wq_d = nc.dram_tensor("Wq", [D, D], f32, kind="ExternalInput").ap()
    bq_d = nc.dram_tensor("bq", [D], f32, kind="ExternalInput").ap()
    Wk_d = nc.dram_tensor("Wk", [D, D], f32, kind="ExternalInput").ap()
    bk_d = nc.dram_tensor("bk", [D], f32, kind="ExternalInput").ap()
    Wv_d = nc.dram_tensor("Wv", [D, D], f32, kind="ExternalInput").ap()
    bv_d = nc.dram_tensor("bv", [D], f32, kind="ExternalInput").ap()
    gamma_d = nc.dram_tensor("gamma", [D], f32, kind="ExternalInput").ap()
    beta_d = nc.dram_tensor("beta", [D], f32, kind="ExternalInput").ap()
    y_d = nc.dram_tensor("y", [S, D], f32, kind="ExternalOutput").ap()

    from contextlib import ExitStack

    with tile.TileContext(nc) as tc, ExitStack() as stk:
      try:
        consts = stk.enter_context(tc.tile_pool(name="consts", bufs=1))
        persist = stk.enter_context(tc.tile_pool(name="persist", bufs=1))
        work = stk.enter_context(tc.tile_pool(name="work", bufs=2))
        small = stk.enter_context(tc.tile_pool(name="small", bufs=6))
        epool = stk.enter_context(tc.tile_pool(name="epool", bufs=4))
        outp = stk.enter_context(tc.tile_pool(name="outp", bufs=3))
        # PSUM (8 banks): shared score/scratch pool 3x[128,1024] = 6 + po 2.
        # 3 bufs let scores(kc+1) stream while exp(kc) drains both heads.
        pool_sc = stk.enter_context(tc.tile_pool(name="psum_sc", bufs=3, space="PSUM"))
        pool_po = stk.enter_context(tc.tile_pool(name="psum_po", bufs=1, space="PSUM"))
        dram = stk.enter_context(tc.tile_pool(name="dram", bufs=1, space="DRAM"))

        # ---------- constants ----------
        ident = consts.tile([P, P], f32)
        make_identity(nc, ident)

        # transposed-layout per-partition vectors [128, ND]
        bqT = consts.tile([P, ND], f32)
        nc.gpsimd.dma_start(out=bqT, in_=bq_d.rearrange("(m p) -> p m", p=P))
        bkT = consts.tile([P, ND], f32)
        nc.gpsimd.dma_start(out=bkT, in_=bk_d.rearrange("(m p) -> p m", p=P))
        gT = consts.tile([P, ND], f32)
        nc.gpsimd.dma_start(out=gT, in_=gamma_d.rearrange("(m p) -> p m", p=P))
        betaT = consts.tile([P, ND], f32)
        nc.gpsimd.dma_start(out=betaT, in_=beta_d.rearrange("(m p) -> p m", p=P))
        # bv per-pair slice broadcast across partitions [128, 128]; the V
        # evacuation repeats it across the 8 s-chunk blocks via a 0-stride AP
        bvb = []
        for p in range(H // 2):
            t = consts.tile([P, P], f32, name=f"bvb{p}")
            bsrc = bass.AP(tensor=bv_d.tensor, offset=bv_d.offset + p * P,
                           ap=[[0, P], [1, P]])
            nc.gpsimd.dma_start(out=t, in_=bsrc)
            bvb.append(t)
        epsT = consts.tile([P, 1], f32)
        nc.vector.memset(epsT, BN_EPS)



        # ---------- load x and weights; cast W to bf16 (Scalar+Vector) -----
        x_nat = []
        for i in range(NS):
            t = persist.tile([P, D], f32, name=f"x_nat{i}", tag=f"x_nat{i}")
            nc.sync.dma_start(out=t, in_=x_d[i * P:(i + 1) * P, :])
            x_nat.append(t)

        # W loads: dense full-row [128, 512] chunks (column-sliced DMAs are
        # ~4x slower), issued on the idle Scalar queue, casts split
        # Scalar/Vector.  wblk() returns the [128,128] column slice.
        wdram = {"q": Wq_d, "k": Wk_d, "v": Wv_d}
        wsb = {}
        for nm in ("q", "k", "v"):
            tiles = []
            for k in range(ND):
                tf = work.tile([P, D], f32, tag="wstage")
                nc.scalar.dma_start(
                    out=tf, in_=wdram[nm][k * P:(k + 1) * P, :])
                t = persist.tile([P, D], bf16, name=f"W{nm}{k}",
                                 tag=f"W{nm}{k}")
                if k % 2 == 0:
                    nc.scalar.copy(t, tf)
                else:
                    nc.vector.tensor_copy(t, tf)
                tiles.append(t)
            wsb[nm] = tiles

        def wblk(nm, k, m):
            return wsb[nm][k][:, m * P:(m + 1) * P]

        def r(ap):
            return ap

        # ---------- transpose x -> xT (4 tiles [128, S] bf16) ----------
        xT = [persist.tile([P, S], bf16, name=f"xT{j}", tag=f"xT{j}")
              for j in range(ND)]
        for j in range(ND):
            pt = pool_sc.tile([P, S], f32, tag="sc")
            for i in range(NS):
                nc.tensor.transpose(
                    pt[:, i * P:(i + 1) * P],
                    x_nat[i][:, j * P:(j + 1) * P],
                    ident,
                )
            nc.vector.tensor_copy(xT[j], pt)

        # ---------- Q^T, K^T (transposed layout, f32r matmuls) ----------
        # Q evac on ScalarE (bias+relu fused), K evac on VectorE (fused
        # tensor_scalar add+max) to keep ScalarE light.
        QT = [persist.tile([P, S], bf16, name=f"QT{m}", tag=f"QT{m}")
              for m in range(ND)]
        KT = [persist.tile([P, S], bf16, name=f"KT{m}", tag=f"KT{m}")
              for m in range(ND)]

        def emit_qk_group(m, g, q_on_scalar=False):
            dst, wname, bT = ((QT, "q", bqT), (KT, "k", bkT))[g // NQ]
            n = g % NQ
            pq = pool_sc.tile([P, S], f32, tag="sc", name="pq")
            pq = pq[:, 0:512]
            for k in range(ND):
                nc.tensor.matmul(
                    pq,
                    lhsT=wblk(wname, k, m),
                    rhs=r(xT[k][:, n * QW:(n + 1) * QW]),
                    start=(k == 0), stop=(k == ND - 1),
                )
            if wname == "q" and q_on_scalar:
                # pre-attention only: ScalarE is idle then
                nc.scalar.activation(
                    out=dst[m][:, n * QW:(n + 1) * QW],
                    in_=pq,
                    func=AF.Relu,
                    bias=bT[:, m:m + 1],
                )
            else:
                nc.vector.tensor_scalar(
                    out=dst[m][:, n * QW:(n + 1) * QW],
                    in0=pq,
                    scalar1=bT[:, m:m + 1],
                    scalar2=0.0,
                    op0=ALU.add,
                    op1=ALU.max,
                )

        def emit_qk(m, q_on_scalar=False):
            for g in range(2 * NQ):
                emit_qk_group(m, g, q_on_scalar)

        # V per PAIR: V_pair[p] [128, 8*128] holds relu-deferred (V+bv) for
        # the pair's 128 e-columns, one [128,128] block per s-chunk kc.
        # Needs only the m=p column block of Wv.
        V_pair = [persist.tile([P, S], f32, name=f"Vp{p}", tag=f"Vp{p}")
                  for p in range(NPAIR)]

        def emit_vpair(p):
            pv = pool_sc.tile([P, S], f32, tag="sc", name="pv")
            for i in range(NS):
                for k in range(ND):
                    nc.tensor.matmul(
                        pv[:, i * P:(i + 1) * P],
                        lhsT=r(xT[k][:, i * P:(i + 1) * P]),
                        rhs=wblk("v", k, p),
                        start=(k == 0), stop=(k == ND - 1),
                    )
            brep = bass.AP(tensor=bvb[p].tensor, offset=bvb[p].offset,
                           ap=[list(bvb[p].ap[0]), [0, NS], [1, P]])
            nc.vector.tensor_add(V_pair[p], pv, brep)

        # pair-0 dependencies first; qk(1..3), V(1..3) and the W column
        # blocks m>=1 interleave into the attention pair stream below (the
        # PE queue is in-order: emitting them early would delay pair 0).
        emit_qk(0, q_on_scalar=True)
        emit_vpair(0)

        if stop_after == "qkv":
            for p in range(1, NPAIR):
                emit_vpair(p)
                emit_qk(p)
            raise _Done()

        # ---------- attention: head pairs, row/col-tiled concurrent MMs ----
        yT = [persist.tile([P, S], f32, name=f"yT{m}", tag=f"yT{m}")
              for m in range(ND)]
        # deferred post-collective tail work, emitted one pair later
        stg2 = [None] * ND
        stg_t = [None] * ND

        def emit_pair(hp):
            m = hp
            hA, hB = 2 * hp, 2 * hp + 1
            QA, KA = QT[m][0:DH, :], KT[m][0:DH, :]
            QB, KB = QT[m][DH:P, :], KT[m][DH:P, :]
            po = pool_po.tile([P, S], f32, tag="po")
            rsA = work.tile([P, NS], f32, tag="rsA")
            rsB = work.tile([P, NS], f32, tag="rsB")
            eA = [None] * NS
            eB = [None] * NS
            vA = [None] * NS
            vB = [None] * NS

            def emit_scores(kc):
                sa = pool_sc.tile([P, S], f32, tag="sc")
                sb = pool_sc.tile([P, S], f32, tag="sc")
                for n in range(NQ):
                    nc.tensor.matmul(
                        sa[:, n * QW:(n + 1) * QW],
                        lhsT=r(KA[:, kc * P:(kc + 1) * P]),
                        rhs=r(QA[:, n * QW:(n + 1) * QW]),
                        start=True, stop=True,
                    )
                    nc.tensor.matmul(
                        sb[:, n * QW:(n + 1) * QW],
                        lhsT=r(KB[:, kc * P:(kc + 1) * P]),
                        rhs=r(QB[:, n * QW:(n + 1) * QW]),
                        start=True, stop=True,
                    )
                ea = epool.tile([P, S], bf16, tag="E")
                nc.scalar.activation(
                    out=ea, in_=sa, func=AF.Exp, scale=inv_sqrt_dh,
                    accum_out=rsA[:, kc:kc + 1])
                eb = epool.tile([P, S], bf16, tag="E")
                nc.scalar.activation(
                    out=eb, in_=sb, func=AF.Exp, scale=inv_sqrt_dh,
                    accum_out=rsB[:, kc:kc + 1])
                eA[kc], eB[kc] = ea, eb
                for rs, vv, j in ((rsA, vA, 0), (rsB, vB, 1)):
                    rr = small.tile([P, 1], f32, tag="rr")
                    nc.vector.reciprocal(rr, rs[:, kc:kc + 1])
                    vp = small.tile([P, DH], bf16, tag="vp")
                    nc.vector.tensor_scalar(
                        out=vp,
                        in0=V_pair[hp][:, kc * P + j * DH:kc * P + (j + 1) * DH],
                        scalar1=0.0, scalar2=rr,
                        op0=ALU.max, op1=ALU.mult,
                    )
                    vv[kc] = vp

            def emit_av(kc):
                for n in range(NQ):
                    nc.tensor.matmul(
                        po[0:DH, n * QW:(n + 1) * QW],
                        lhsT=vA[kc],
                        rhs=eA[kc][:, n * QW:(n + 1) * QW],
                        start=(kc == 0), stop=(kc == NS - 1),
                    )
                    nc.tensor.matmul(
                        po[DH:P, n * QW:(n + 1) * QW],
                        lhsT=vB[kc],
                        rhs=eB[kc][:, n * QW:(n + 1) * QW],
                        start=(kc == 0), stop=(kc == NS - 1),
                    )

            for kc in range(NS):
                emit_scores(kc)
                if hp + 1 < NPAIR and 2 <= kc <= 5:
                    # spread next pair's projections: one (dst, n) group per
                    # chunk instead of a 16-matmul burst at the boundary
                    emit_qk_group(hp + 1, kc - 2)
                if hp + 1 < NPAIR and kc == 6:
                    emit_vpair(hp + 1)
                if kc >= 1:
                    emit_av(kc - 1)
            emit_av(NS - 1)

            # residual into yT[m], local stats (bn_stats), async AllReduce
            nc.vector.tensor_add(yT[m], po, xT[m])
            st6 = work.tile([P, 12], f32, tag="st6")
            nc.vector.bn_stats(st6[:, 0:6], yT[m][:, 0:512])
            nc.vector.bn_stats(st6[:, 6:12], yT[m][:, 512:1024])
            mvl = work.tile([P, 2], f32, tag="mvl")
            nc.vector.bn_aggr(mvl, st6)
            # convert (mean, var) -> (sum, sumsq) for the additive AllReduce
            st = work.tile([P, 2], f32, tag="st")
            m2l = small.tile([P, 1], f32, tag="m2l")
            nc.vector.tensor_mul(m2l, mvl[:, 0:1], mvl[:, 0:1])
            nc.vector.tensor_scalar_mul(st[:, 0:1], mvl[:, 0:1], float(S))
            nc.vector.tensor_add(m2l, m2l, mvl[:, 1:2])
            nc.vector.tensor_scalar_mul(st[:, 1:2], m2l, float(S))

            stg = consts.tile([P, 2], f32, name=f"stg{m}")
            if use_collective:
                stats_in = dram.tile([P, 2], f32, tag=f"ci{m}")
                stats_out = dram.tile(
                    [P, 2], f32, tag=f"co{m}",
                    addr_space="Shared" if n_cores > 4 else "Local")
                nc.gpsimd.dma_start(out=stats_in, in_=st)
                nc.gpsimd.collective_compute(
                    "AllReduce",
                    ALU.add,
                    replica_groups=[list(range(n_cores))],
                    ins=[stats_in.opt()],
                    outs=[stats_out.opt()],
                )
                nc.sync.dma_start(out=stg, in_=stats_out)
            else:
                nc.vector.tensor_scalar_mul(stg, st, float(n_cores))
            stg2[m] = stg

        def emit_tail(m):
            # post-collective: BN coefficients on VectorE only (keeps ScalarE
            # in the exp table set - no sqrt table load). rsqrt via bit-hack
            # seed + 2 Newton iterations; [128,1] ops are ~70ns each.
            stg = stg2[m]
            mv = consts.tile([P, 2], f32, name=f"mv{m}")
            nc.vector.tensor_scalar_mul(mv, stg, inv_ntok)  # [E[y], E[y^2]]
            m2 = small.tile([P, 1], f32, tag="m2")
            nc.vector.tensor_mul(m2, mv[:, 0:1], mv[:, 0:1])
            var = small.tile([P, 1], f32, tag="var")
            nc.vector.tensor_scalar(
                out=var, in0=m2, scalar1=-1.0, scalar2=BN_EPS,
                op0=ALU.mult, op1=ALU.add)
            nc.vector.tensor_add(var, var, mv[:, 1:2])  # var+eps, biased
            i32 = mybir.dt.int32
            si = small.tile([P, 1], i32, tag="si")
            nc.vector.tensor_scalar(
                out=si, in0=var.bitcast(i32), scalar1=1, scalar2=None,
                op0=ALU.logical_shift_right)
            mg = small.tile([P, 1], i32, tag="mg")
            nc.vector.tensor_scalar(
                out=mg, in0=si, scalar1=-1, scalar2=0x5F3759DF,
                op0=ALU.mult, op1=ALU.add)
            y0 = mg.bitcast(f32)
            rsd = small.tile([P, 1], f32, tag="rsd")
            t = small.tile([P, 1], f32, tag="nt")
            for it in range(2):
                src = y0 if it == 0 else rsd
                nc.vector.tensor_mul(t, src, src)
                nc.vector.tensor_mul(t, t, var)
                nc.vector.tensor_scalar(
                    out=t, in0=t, scalar1=-0.5, scalar2=1.5,
                    op0=ALU.mult, op1=ALU.add)
                nc.vector.tensor_mul(rsd, src, t)
            A = consts.tile([P, 1], f32, name=f"A{m}")
            nc.vector.tensor_mul(A, gT[:, m:m + 1], rsd)
            C = consts.tile([P, 1], f32, name=f"C{m}")
            nc.vector.tensor_mul(C, mv[:, 0:1], A)
            nc.vector.tensor_sub(C, betaT[:, m:m + 1], C)
            # affine on VectorE (fused mult+add), f32
            z = work.tile([P, S], f32, tag="z")
            nc.vector.tensor_scalar(
                out=z, in0=yT[m],
                scalar1=A, scalar2=C,
                op0=ALU.mult, op1=ALU.add,
            )
            # transpose column m back to natural layout, DMA out per block
            pz = pool_sc.tile([P, S], f32, tag="sc")
            for i in range(NS):
                nc.tensor.transpose(
                    pz[:, i * P:(i + 1) * P],
                    z[:, i * P:(i + 1) * P],
                    ident,
                )
            oc = outp.tile([P, S], f32, tag="yo")
            nc.vector.tensor_copy(oc, pz)
            for i in range(NS):
                nc.sync.dma_start(
                    out=y_d[i * P:(i + 1) * P, m * P:(m + 1) * P],
                    in_=oc[:, i * P:(i + 1) * P])

        # tails lag their pair by TWO pairs so the AllReduce has ~60us to
        # resolve (covers launch skew) before its consumers reach the
        # in-order engine queues; only tails 2 and 3 are exposed at the end.
        for hp in range(NPAIR):
            emit_pair(hp)
            if hp >= 2:
                emit_tail(hp - 2)
            if stop_after == "attn" and hp == NPAIR - 1:
                raise _Done()
        emit_tail(NPAIR - 2)
        emit_tail(NPAIR - 1)
      except _Done:
        pass

    nc.compile()
    return nc


def _get_program(S=S_FULL, n_cores=N_CORES, total_tokens=None):
    key = (S, n_cores, total_tokens)
    if key not in _CACHE:
        _CACHE[key] = _build(S, n_cores, total_tokens)
    return _CACHE[key]


def kernel(**inputs):
    x = np.ascontiguousarray(np.asarray(inputs["x"], dtype=np.float32))
    B, S, Dx = x.shape
    assert (B, S, Dx) == (B_FULL, S_FULL, D), (B, S, Dx)
    names = ["Wq", "bq", "Wk", "bk", "Wv", "bv", "gamma", "beta"]
    shared = {
        n: np.ascontiguousarray(np.asarray(inputs[n], dtype=np.float32))
        for n in names
    }

    nc = _get_program()
    in_maps = [dict(shared, x=x[c]) for c in range(N_CORES)]

    from concourse.bass_utils import run_bass_kernel_spmd
    res = run_bass_kernel_spmd(nc, in_maps, core_ids=list(range(N_CORES)))
    y = np.stack([res.results[c]["y"] for c in range(N_CORES)], axis=0)
    return y.astype(np.float32)


if __name__ == "__main__":
    rng = np.random.default_rng(0)
    demo = {
        "x": rng.standard_normal((B_FULL, S_FULL, D), dtype=np.float32),
        "Wq": rng.standard_normal((D, D), dtype=np.float32) * 0.02,
        "bq": np.zeros(D, np.float32),
        "Wk": rng.standard_normal((D, D), dtype=np.float32) * 0.02,
        "bk": np.zeros(D, np.float32),
        "Wv": rng.standard_normal((D, D), dtype=np.float32) * 0.02,
        "bv": np.zeros(D, np.float32),
        "gamma": np.ones(D, np.float32),
        "beta": np.zeros(D, np.float32),
    }
    out = kernel(**demo)
    print("kernel output", out.shape, out.dtype, float(np.abs(out).max()))


# revision 36
# speedup vs baseline: 1.3451x; 1.3451x over previous
"""Trainium2 Bass kernel for nn_MultiHeadAttention_56066503082144.

Reference computation (per batch b):
  Q = relu(x @ Wq + bq), K = relu(x @ Wk + bk), V = relu(x @ Wv + bv)
  scores[b,h,q,k] = (Q_h @ K_h^T) / sqrt(dh)
  attn = softmax(scores, axis=q)            # NON-STANDARD: over the query axis
  out[b,q,:] = concat_h(attn_h @ V_h)
  y = out + x                               # residual
  y = batchnorm(y)                          # per-channel stats over (B, S)

Sharding: data-parallel over batch B=8 across the 8 NeuronCores (one batch
element per core).  Cross-core communication: four 1KB AllReduces of
per-d-chunk BatchNorm partial sums, pipelined under the attention compute.

v2 design notes (trace-driven):
  - All projection/score matmuls run in float32r (TF32-like, 1 cycle/row at
    N=512) directly on f32 tiles - no bf16 weight casts at all.
  - Attention processes HEAD PAIRS: the two heads of a 128-partition Q/K tile
    occupy partitions 0:64 / 64:128, so their score matmuls auto-derive
    tile_position (0,0) / (64,0) (row groups) and run CONCURRENTLY on the PE's
    independent 64-row sub-arrays.  attn@V similarly col-tiles into po[0:64]
    and po[64:128] (tile_position (0,0) / (0,64)).
  - ScalarE is the pipeline pacer: 64 exp ACTIVATEs of [128,1024] (~1.02us
    each) with fused accum for softmax row sums.  Everything else is kept off
    ScalarE (K/V evacuation and the BN affine live on VectorE).
  - Softmax denominator folded into V rows (V' = relu(V+bv) * 1/rowsum as a
    single fused tensor_scalar) instead of rescaling the big E matrix.
  - BN tail pipelined per d-chunk m: residual+stats+AllReduce emitted with
    pair m, the post-collective ops (coeffs, affine, transpose-back, DMA out)
    emitted one pair LATER so in-order engine queues never stall on the
    collective.  Only chunk m=3's tail is exposed at the end.
  - rsd = exp(-0.5*ln(var+eps)): keeps every activation in the single
    natural_log_exp_and_others table set (no sqrt table load).  A dummy ln at
    kernel start pins that set.
"""

import math

import numpy as np

P = 128
D = 512
H = 8
DH = 64
S_FULL = 1024
B_FULL = 8
N_CORES = 8
BN_EPS = 1e-5

_CACHE = {}


class _Done(Exception):
    """Early-exit marker for phase-bisection builds."""


def _build(S=S_FULL, n_cores=N_CORES, total_tokens=None, stop_after="full",
           use_collective=True):
    import concourse.bacc as bacc
    import concourse.bass as bass
    import concourse.tile as tile
    from concourse import mybir
    from concourse.masks import make_identity

    f32 = mybir.dt.float32
    f32r = mybir.dt.float32r
    bf16 = mybir.dt.bfloat16
    AF = mybir.ActivationFunctionType
    ALU = mybir.AluOpType
    AX = mybir.AxisListType

    if total_tokens is None:
        total_tokens = n_cores * S
    inv_ntok = 1.0 / float(total_tokens)

    ND = D // P          # 4 d-chunks
    NS = S // P          # 8 s-chunks
    QW = min(512, S)     # matmul moving-operand tile
    NQ = S // QW         # 2
    NPAIR = H // 2       # 4 head pairs (pair hp lives in QT[hp]/KT[hp])
    inv_sqrt_dh = 1.0 / math.sqrt(DH)

    nc = bacc.Bacc(
        "TRN2",
        target_bir_lowering=False,
        debug=False,
        num_devices=n_cores,
    )

    x_d = nc.dram_tensor("x", [S, D], f32, kind="ExternalInput").ap()
    Wq_d = nc.dram_tensor("Wq", [D, D], f32, kind="ExternalInput").ap()
    bq_d = nc.dram_tensor("bq", [D], f32, kind="ExternalInput").ap()
    Wk_d = nc.dram_tensor("Wk", [D, D], f32, kind="ExternalInput").ap()
    bk_d = nc.dram_tensor("bk", [D], f32, kind="ExternalInput").ap()
    Wv_d = nc.dram_tensor("Wv", [D, D], f32, kind="ExternalInput").ap()
    bv_d = nc.dram_tensor("bv", [D], f32, kind="ExternalInput").ap()
    gamma_d = nc.dram_tensor("gamma", [D], f32, kind="ExternalInput").ap()
    beta_d = nc.dram_tensor("beta", [D], f32, kind="ExternalInput").ap()
    y_d = nc.dram_tensor("y", [S, D], f32, kind="ExternalOutput").ap()

    from contextlib import ExitStack

    with tile.TileContext(nc) as tc, ExitStack() as stk:
      try:
        consts = stk.enter_context(tc.tile_pool(name="consts", bufs=1))
        persist = stk.enter_context(tc.tile_pool(name="persist", bufs=1))
        work = stk.enter_context(tc.tile_pool(name="work", bufs=2))
        small = stk.enter_context(tc.tile_pool(name="small", bufs=6))
        epool = stk.enter_context(tc.tile_pool(name="epool", bufs=4))
        outp = stk.enter_context(tc.tile_pool(name="outp", bufs=3))
        # PSUM (8 banks): shared score/scratch pool 3x[128,1024] = 6 + po 2.
        # 3 bufs let scores(kc+1) stream while exp(kc) drains both heads.
        pool_sc = stk.enter_context(tc.tile_pool(name="psum_sc", bufs=3, space="PSUM"))
        pool_po = stk.enter_context(tc.tile_pool(name="psum_po", bufs=1, space="PSUM"))
        dram = stk.enter_context(tc.tile_pool(name="dram", bufs=1, space="DRAM"))

        # ---------- constants ----------
        ident = consts.tile([P, P], f32)
        make_identity(nc, ident)

        # transposed-layout per-partition vectors [128, ND]
        bqT = consts.tile([P, ND], f32)
        nc.gpsimd.dma_start(out=bqT, in_=bq_d.rearrange("(m p) -> p m", p=P))
        bkT = consts.tile([P, ND], f32)
        nc.gpsimd.dma_start(out=bkT, in_=bk_d.rearrange("(m p) -> p m", p=P))
        gT = consts.tile([P, ND], f32)
        nc.gpsimd.dma_start(out=gT, in_=gamma_d.rearrange("(m p) -> p m", p=P))
        betaT = consts.tile([P, ND], f32)
        nc.gpsimd.dma_start(out=betaT, in_=beta_d.rearrange("(m p) -> p m", p=P))
        # bv per-pair slice broadcast across partitions [128, 128]; the V
        # evacuation repeats it across the 8 s-chunk blocks via a 0-stride AP
        bvb = []
        for p in range(H // 2):
            t = consts.tile([P, P], f32, name=f"bvb{p}")
            bsrc = bass.AP(tensor=bv_d.tensor, offset=bv_d.offset + p * P,
                           ap=[[0, P], [1, P]])
            nc.gpsimd.dma_start(out=t, in_=bsrc)
            bvb.append(t)
        epsT = consts.tile([P, 1], f32)
        nc.vector.memset(epsT, BN_EPS)



        # ---------- load x and weights; cast W to bf16 (Scalar+Vector) -----
        x_nat = []
        for i in range(NS):
            t = persist.tile([P, D], f32, name=f"x_nat{i}", tag=f"x_nat{i}")
            nc.sync.dma_start(out=t, in_=x_d[i * P:(i + 1) * P, :])
            x_nat.append(t)

        # W loads: dense full-row [128, 512] chunks (column-sliced DMAs are
        # ~4x slower), issued on the idle Scalar queue, casts split
        # Scalar/Vector.  wblk() returns the [128,128] column slice.
        wdram = {"q": Wq_d, "k": Wk_d, "v": Wv_d}
        wsb = {}
        for nm in ("q", "k", "v"):
            tiles = []
            for k in range(ND):
                tf = work.tile([P, D], f32, tag="wstage")
                nc.scalar.dma_start(
                    out=tf, in_=wdram[nm][k * P:(k + 1) * P, :])
                t = persist.tile([P, D], bf16, name=f"W{nm}{k}",
                                 tag=f"W{nm}{k}")
                if k % 2 == 0:
                    nc.scalar.copy(t, tf)
                else:
                    nc.vector.tensor_copy(t, tf)
                tiles.append(t)
            wsb[nm] = tiles

        def wblk(nm, k, m):
            return wsb[nm][k][:, m * P:(m + 1) * P]

        def r(ap):
            return ap

        # ---------- transpose x -> xT (4 tiles [128, S] bf16) ----------
        xT = [persist.tile([P, S], bf16, name=f"xT{j}", tag=f"xT{j}")
              for j in range(ND)]
        for j in range(ND):
            pt = pool_sc.tile([P, S], f32, tag="sc")
            for i in range(NS):
                nc.tensor.transpose(
                    pt[:, i * P:(i + 1) * P],
                    x_nat[i][:, j * P:(j + 1) * P],
                    ident,
                )
            nc.vector.tensor_copy(xT[j], pt)

        # ---------- Q^T, K^T (transposed layout, f32r matmuls) ----------
        # Q evac on ScalarE (bias+relu fused), K evac on VectorE (fused
        # tensor_scalar add+max) to keep ScalarE light.
        QT = [persist.tile([P, S], bf16, name=f"QT{m}", tag=f"QT{m}")
              for m in range(ND)]
        KT = [persist.tile([P, S], bf16, name=f"KT{m}", tag=f"KT{m}")
              for m in range(ND)]

        def emit_qk_group(m, g, q_on_scalar=False):
            dst, wname, bT = ((QT, "q", bqT), (KT, "k", bkT))[g // NQ]
            n = g % NQ
            pq = pool_sc.tile([P, S], f32, tag="sc", name="pq")
            pq = pq[:, 0:512]
            for k in range(ND):
                nc.tensor.matmul(
                    pq,
                    lhsT=wblk(wname, k, m),
                    rhs=r(xT[k][:, n * QW:(n + 1) * QW]),
                    start=(k == 0), stop=(k == ND - 1),
                )
            if wname == "q" and q_on_scalar:
                # pre-attention only: ScalarE is idle then
                nc.scalar.activation(
                    out=dst[m][:, n * QW:(n + 1) * QW],
                    in_=pq,
                    func=AF.Relu,
                    bias=bT[:, m:m + 1],
                )
            else:
                nc.vector.tensor_scalar(
                    out=dst[m][:, n * QW:(n + 1) * QW],
                    in0=pq,
                    scalar1=bT[:, m:m + 1],
                    scalar2=0.0,
                    op0=ALU.add,
                    op1=ALU.max,
                )

        def emit_qk(m, q_on_scalar=False):
            for g in range(2 * NQ):
                emit_qk_group(m, g, q_on_scalar)

        # V per PAIR: V_pair[p] [128, 8*128] holds relu-deferred (V+bv) for
        # the pair's 128 e-columns, one [128,128] block per s-chunk kc.
        # Needs only the m=p column block of Wv.
        V_pair = [persist.tile([P, S], f32, name=f"Vp{p}", tag=f"Vp{p}")
                  for p in range(NPAIR)]

        def emit_vpair(p):
            pv = pool_sc.tile([P, S], f32, tag="sc", name="pv")
            for i in range(NS):
                for k in range(ND):
                    nc.tensor.matmul(
                        pv[:, i * P:(i + 1) * P],
                        lhsT=r(xT[k][:, i * P:(i + 1) * P]),
                        rhs=wblk("v", k, p),
                        start=(k == 0), stop=(k == ND - 1),
                    )
            brep = bass.AP(tensor=bvb[p].tensor, offset=bvb[p].offset,
                           ap=[list(bvb[p].ap[0]), [0, NS], [1, P]])
            nc.vector.tensor_add(V_pair[p], pv, brep)

        # pair-0 dependencies first; qk(1..3), V(1..3) and the W column
        # blocks m>=1 interleave into the attention pair stream below (the
        # PE queue is in-order: emitting them early would delay pair 0).
        emit_qk(0, q_on_scalar=True)
        emit_vpair(0)

        if stop_after == "qkv":
            for p in range(1, NPAIR):
                emit_vpair(p)
                emit_qk(p)
            raise _Done()

        # ---------- attention: head pairs, row/col-tiled concurrent MMs ----
        yT = [persist.tile([P, S], f32, name=f"yT{m}", tag=f"yT{m}")
              for m in range(ND)]
        # deferred post-collective tail work, emitted one pair later
        stg2 = [None] * ND
        stg_t = [None] * ND

        def emit_pair(hp):
            m = hp
            hA, hB = 2 * hp, 2 * hp + 1
            QA, KA = QT[m][0:DH, :], KT[m][0:DH, :]
            QB, KB = QT[m][DH:P, :], KT[m][DH:P, :]
            po = pool_po.tile([P, S], f32, tag="po")
            rsA = work.tile([P, NS], f32, tag="rsA")
            rsB = work.tile([P, NS], f32, tag="rsB")
            eA = [None] * NS
            eB = [None] * NS
            vA = [None] * NS
            vB = [None] * NS

            def emit_scores(kc):
                sa = pool_sc.tile([P, S], f32, tag="sc")
                sb = pool_sc.tile([P, S], f32, tag="sc")
                for n in range(NQ):
                    nc.tensor.matmul(
                        sa[:, n * QW:(n + 1) * QW],
                        lhsT=r(KA[:, kc * P:(kc + 1) * P]),
                        rhs=r(QA[:, n * QW:(n + 1) * QW]),
                        start=True, stop=True,
                    )
                    nc.tensor.matmul(
                        sb[:, n * QW:(n + 1) * QW],
                        lhsT=r(KB[:, kc * P:(kc + 1) * P]),
                        rhs=r(QB[:, n * QW:(n + 1) * QW]),
                        start=True, stop=True,
                    )
                ea = epool.tile([P, S], bf16, tag="E")
                nc.scalar.activation(
                    out=ea, in_=sa, func=AF.Exp, scale=inv_sqrt_dh,
                    accum_out=rsA[:, kc:kc + 1])
                eb = epool.tile([P, S], bf16, tag="E")
                nc.scalar.activation(
                    out=eb, in_=sb, func=AF.Exp, scale=inv_sqrt_dh,
                    accum_out=rsB[:, kc:kc + 1])
                eA[kc], eB[kc] = ea, eb
                for rs, vv, j in ((rsA, vA, 0), (rsB, vB, 1)):
                    rr = small.tile([P, 1], f32, tag="rr")
                    nc.vector.reciprocal(rr, rs[:, kc:kc + 1])
                    vp = small.tile([P, DH], bf16, tag="vp")
                    nc.vector.tensor_scalar(
                        out=vp,
                        in0=V_pair[hp][:, kc * P + j * DH:kc * P + (j + 1) * DH],
                        scalar1=0.0, scalar2=rr,
                        op0=ALU.max, op1=ALU.mult,
                    )
                    vv[kc] = vp

            def emit_av(kc):
                for n in range(NQ):
                    nc.tensor.matmul(
                        po[0:DH, n * QW:(n + 1) * QW],
                        lhsT=vA[kc],
                        rhs=eA[kc][:, n * QW:(n + 1) * QW],
                        start=(kc == 0), stop=(kc == NS - 1),
                    )
                    nc.tensor.matmul(
                        po[DH:P, n * QW:(n + 1) * QW],
                        lhsT=vB[kc],
                        rhs=eB[kc][:, n * QW:(n + 1) * QW],
                        start=(kc == 0), stop=(kc == NS - 1),
                    )

            for kc in range(NS):
                emit_scores(kc)
                if hp + 1 < NPAIR and 2 <= kc <= 5:
                    # spread next pair's projections: one (dst, n) group per
                    # chunk instead of a 16-matmul burst at the boundary
                    emit_qk_group(hp + 1, kc - 2)
                if hp + 1 < NPAIR and kc == 6:
                    emit_vpair(hp + 1)
                if kc >= 1:
                    emit_av(kc - 1)
            emit_av(NS - 1)

            # residual into yT[m], local stats (bn_stats), async AllReduce
            nc.vector.tensor_add(yT[m], po, xT[m])
            st6 = work.tile([P, 12], f32, tag="st6")
            nc.vector.bn_stats(st6[:, 0:6], yT[m][:, 0:512])
            nc.vector.bn_stats(st6[:, 6:12], yT[m][:, 512:1024])
            mvl = work.tile([P, 2], f32, tag="mvl")
            nc.vector.bn_aggr(mvl, st6)
            # convert (mean, var) -> (sum, sumsq) for the additive AllReduce
            st = work.tile([P, 2], f32, tag="st")
            m2l = small.tile([P, 1], f32, tag="m2l")
            nc.vector.tensor_mul(m2l, mvl[:, 0:1], mvl[:, 0:1])
            nc.vector.tensor_scalar_mul(st[:, 0:1], mvl[:, 0:1], float(S))
            nc.vector.tensor_add(m2l, m2l, mvl[:, 1:2])
            nc.vector.tensor_scalar_mul(st[:, 1:2], m2l, float(S))

            stg = consts.tile([P, 2], f32, name=f"stg{m}")
            if use_collective:
                stats_in = dram.tile([P, 2], f32, tag=f"ci{m}")
                stats_out = dram.tile(
                    [P, 2], f32, tag=f"co{m}",
                    addr_space="Shared" if n_cores > 4 else "Local")
                nc.gpsimd.dma_start(out=stats_in, in_=st)
                nc.gpsimd.collective_compute(
                    "AllReduce",
                    ALU.add,
                    replica_groups=[list(range(n_cores))],
                    ins=[stats_in.opt()],
                    outs=[stats_out.opt()],
                )
                nc.sync.dma_start(out=stg, in_=stats_out)
            else:
                nc.vector.tensor_scalar_mul(stg, st, float(n_cores))
            stg2[m] = stg

        def emit_tail(m):
            # post-collective: BN coefficients on VectorE only (keeps ScalarE
            # in the exp table set - no sqrt table load). rsqrt via bit-hack
            # seed + 2 Newton iterations; [128,1] ops are ~70ns each.
            stg = stg2[m]
            mv = consts.tile([P, 2], f32, name=f"mv{m}")
            nc.vector.tensor_scalar_mul(mv, stg, inv_ntok)  # [E[y], E[y^2]]
            m2 = small.tile([P, 1], f32, tag="m2")
            nc.vector.tensor_mul(m2, mv[:, 0:1], mv[:, 0:1])
            var = small.tile([P, 1], f32, tag="var")
            nc.vector.tensor_scalar(
                out=var, in0=m2, scalar1=-1.0, scalar2=BN_EPS,
                op0=ALU.mult, op1=ALU.add)
            nc.vector.tensor_add(var, var, mv[:, 1:2])  # var+eps, biased
            i32 = mybir.dt.int32
            si = small.tile([P, 1], i32, tag="si")
            nc.vector.tensor_scalar(
                out=si, in0=var.bitcast(i32), scalar1=1, scalar2=None,
                op0=ALU.logical_shift_right)
            mg = small.tile([P, 1], i32, tag="mg")
            nc.vector.tensor_scalar(
                out=mg, in0=si, scalar1=-1, scalar2=0x5F3759DF,
                op0=ALU.mult, op1=ALU.add)
            y0 = mg.bitcast(f32)
            rsd = small.tile([P, 1], f32, tag="rsd")
            t = small.tile([P, 1], f32, tag="nt")
            for it in range(2):
                src = y0 if it == 0 else rsd
                nc.vector.tensor_mul(t, src, src)
                nc.vector.tensor_mul(t, t, var)
                nc.vector.tensor_scalar(
                    out=t, in0=t, scalar1=-0.5, scalar2=1.5,
                    op0=ALU.mult, op1=ALU.add)
                nc.vector.tensor_mul(rsd, src, t)
            A = consts.tile([P, 1], f32, name=f"A{m}")
            nc.vector.tensor_mul(A, gT[:, m:m + 1], rsd)
            C = consts.tile([P, 1], f32, name=f"C{m}")
            nc.vector.tensor_mul(C, mv[:, 0:1], A)
            nc.vector.tensor_sub(C, betaT[:, m:m + 1], C)
            # affine on ScalarE (idle after attention; Identity is in the
            # exp table set)
            z = work.tile([P, S], f32, tag="z")
            nc.scalar.activation(
                out=z, in_=yT[m], func=AF.Identity,
                bias=C, scale=A,
            )
            # transpose column m back to natural layout, DMA out per block
            pz = pool_sc.tile([P, S], f32, tag="sc")
            for i in range(NS):
                nc.tensor.transpose(
                    pz[:, i * P:(i + 1) * P],
                    z[:, i * P:(i + 1) * P],
                    ident,
                )
            oc = outp.tile([P, S], f32, tag="yo")
            nc.vector.tensor_copy(oc, pz)
            for i in range(NS):
                nc.sync.dma_start(
                    out=y_d[i * P:(i + 1) * P, m * P:(m + 1) * P],
                    in_=oc[:, i * P:(i + 1) * P])

        # ALL post-collective consumption happens after the last pair: the
        # AllReduces are posted per-pair (early) but their results are only
        # read here, so inter-core skew can never stall the attention
        # pipeline mid-kernel.  Results for m<=2 are long ready; only
        # collective 3's latency is exposed.
        for hp in range(NPAIR):
            emit_pair(hp)
            if stop_after == "attn" and hp == NPAIR - 1:
                raise _Done()
        for m in range(ND):
            emit_tail(m)
      except _Done:
        pass

    nc.compile()
    return nc


def _get_program(S=S_FULL, n_cores=N_CORES, total_tokens=None):
    key = (S, n_cores, total_tokens)
    if key not in _CACHE:
        _CACHE[key] = _build(S, n_cores, total_tokens)
    return _CACHE[key]


def kernel(**inputs):
    x = np.ascontiguousarray(np.asarray(inputs["x"], dtype=np.float32))
    B, S, Dx = x.shape
    assert (B, S, Dx) == (B_FULL, S_FULL, D), (B, S, Dx)
    names = ["Wq", "bq", "Wk", "bk", "Wv", "bv", "gamma", "beta"]
    shared = {
        n: np.ascontiguousarray(np.asarray(inputs[n], dtype=np.float32))
        for n in names
    }

    nc = _get_program()
    in_maps = [dict(shared, x=x[c]) for c in range(N_CORES)]

    from concourse.bass_utils import run_bass_kernel_spmd
    res = run_bass_kernel_spmd(nc, in_maps, core_ids=list(range(N_CORES)))
    y = np.stack([res.results[c]["y"] for c in range(N_CORES)], axis=0)
    return y.astype(np.float32)


if __name__ == "__main__":
    rng = np.random.default_rng(0)
    demo = {
        "x": rng.standard_normal((B_FULL, S_FULL, D), dtype=np.float32),
        "Wq": rng.standard_normal((D, D), dtype=np.float32) * 0.02,
        "bq": np.zeros(D, np.float32),
        "Wk": rng.standard_normal((D, D), dtype=np.float32) * 0.02,
        "bk": np.zeros(D, np.float32),
        "Wv": rng.standard_normal((D, D), dtype=np.float32) * 0.02,
        "bv": np.zeros(D, np.float32),
        "gamma": np.ones(D, np.float32),
        "beta": np.zeros(D, np.float32),
    }
    out = kernel(**demo)
    print("kernel output", out.shape, out.dtype, float(np.abs(out).max()))
